# revision 4
# baseline (speedup 1.0000x reference)
"""Multi-head attention (B=2, T=2048, H=8, K=128) on 8 TRN2 NeuronCores.

Sharding: tensor-parallel over heads — core c owns head c for both batches.
Each core computes its head's attention output projected through its slice
of Wu (a partial sum over the unified dim); the host sums the 8 partials
and adds the bias.

Per-core dataflow (everything "transposed": features on partitions, tokens
on the moving/free axis):
  X^T  [k=128, t=4096]   via PE transposes of 32 [128,128] tiles of x
  Q^T  = Wq_h^T X^T      [i=128, 4096]   (fp32r matmul, k-contraction)
  K^T  = Wk_h^T X^T      [i=128, 4096]
  V^T  = Wv_h^T X^T      [j=128, 4096], then PE-transposed back to
  V    [s=128-chunks, j=128]             (lhsT for the Y^T matmul)
  per batch b, per 512-token block t:
    per 128-key chunk s:
      S^T_s = K_s Q^T        [128, 512] PSUM      (QK^T, transposed)
      E_s   = exp(S^T_s/sqrt(128))  ACT, PSUM->SBUF
      sumexp += ones^T E_s   [128, 512] PSUM      (replicated over partitions)
      Y^T   += V_s^T E_s     [128, 512] PSUM
    Y^T_norm = Y^T * recip(sumexp)   DVE, -> SBUF
  out^T = Wu_h^T Y^T_norm   [o=128, 4096] -> DRAM

Host: out = sum_c out_c^T.T + bu, reshaped to (2, 2048, 128).
"""

import sys

import numpy as np

if "/opt/trn_rl_repo" not in sys.path:
    sys.path.insert(0, "/opt/trn_rl_repo")

B, T, K, H = 2, 2048, 128, 8
BT = B * T              # 4096 tokens over both batches
NT = BT // 128          # 32 token tiles of 128
NCORES = 8
TB = 512                # token block (psum moving size)
NS = T // 128           # 16 key chunks per batch
SCALE = 1.0 / np.sqrt(np.float32(K))

_compiled = None


def _build():
    import concourse.mybir as mybir
    import concourse.tile as tile
    from concourse import bacc
    from concourse.masks import make_identity

    f32 = mybir.dt.float32
    f32r = mybir.dt.float32r
    Exp = mybir.ActivationFunctionType.Exp

    nc = bacc.Bacc(
        "TRN2",
        target_bir_lowering=False,
        debug=False,
        enable_asserts=False,
        num_devices=NCORES,
    )

    x_d = nc.dram_tensor("x", [BT, K], f32, kind="ExternalInput").ap()
    wq_d = nc.dram_tensor("wq", [K, K], f32, kind="ExternalInput").ap()
    wk_d = nc.dram_tensor("wk", [K, K], f32, kind="ExternalInput").ap()
    wv_d = nc.dram_tensor("wv", [K, K], f32, kind="ExternalInput").ap()
    wu_d = nc.dram_tensor("wu", [K, K], f32, kind="ExternalInput").ap()
    out_d = nc.dram_tensor("out", [K, BT], f32, kind="ExternalOutput").ap()

    with tile.TileContext(nc) as tc:
        from contextlib import ExitStack

        with ExitStack() as ctx:
            const = ctx.enter_context(tc.tile_pool(name="const", bufs=1))
            big = ctx.enter_context(tc.tile_pool(name="big", bufs=1))
            work = ctx.enter_context(tc.tile_pool(name="work", bufs=3))
            ps_mm = ctx.enter_context(tc.tile_pool(name="ps_mm", bufs=2, space="PSUM"))
            ps_s = ctx.enter_context(tc.tile_pool(name="ps_s", bufs=2, space="PSUM"))
            ps_y = ctx.enter_context(tc.tile_pool(name="ps_y", bufs=2, space="PSUM"))
            ps_sum = ctx.enter_context(tc.tile_pool(name="ps_sum", bufs=2, space="PSUM"))

            ident = const.tile([128, 128], f32)
            make_identity(nc, ident[:])
            ones_st = const.tile([128, 128], f32, tag="ones_st")
            nc.gpsimd.memset(ones_st[:], 1.0)
            ones = const.tile([128, 128], f32r)
            nc.vector.tensor_copy(ones[:], ones_st[:])

            # weights: DMA fp32, then DVE cast-copy to fp32r (the verifier
            # requires fp32r matmul operands to be rounded by a compute
            # engine write)
            wq_st = const.tile([128, 128], f32, tag="wq_st")
            wk_st = const.tile([128, 128], f32, tag="wk_st")
            wv_st = const.tile([128, 128], f32, tag="wv_st")
            wu_st = const.tile([128, 128], f32, tag="wu_st")
            nc.sync.dma_start(wq_st[:], wq_d[:])
            nc.sync.dma_start(wk_st[:], wk_d[:])
            nc.sync.dma_start(wv_st[:], wv_d[:])
            nc.sync.dma_start(wu_st[:], wu_d[:])
            wq_sb = const.tile([128, 128], f32r, tag="wq")
            wk_sb = const.tile([128, 128], f32r, tag="wk")
            wv_sb = const.tile([128, 128], f32r, tag="wv")
            wu_sb = const.tile([128, 128], f32r, tag="wu")
            nc.vector.tensor_copy(wq_sb[:], wq_st[:])
            nc.vector.tensor_copy(wk_sb[:], wk_st[:])
            nc.vector.tensor_copy(wv_sb[:], wv_st[:])
            nc.vector.tensor_copy(wu_sb[:], wu_st[:])

            # x, tiled [p=128, n=32, k=128]; x_sb[p, n, k] = x[n*128+p, k]
            x_sb = big.tile([128, NT, 128], f32, tag="x")
            x_re = x_d.rearrange("(n p) k -> p n k", p=128)
            for h in range(4):
                nc.sync.dma_start(x_sb[:, 8 * h : 8 * (h + 1), :],
                                  x_re[:, 8 * h : 8 * (h + 1), :])

            # X^T [k, t]
            xt = big.tile([128, BT], f32r, tag="xt")
            for n in range(NT):
                pt = ps_mm.tile([128, 128], f32, tag="mm")
                nc.tensor.transpose(pt[:], x_sb[:, n, :], ident[:])
                nc.vector.tensor_copy(xt[:, 128 * n : 128 * (n + 1)], pt[:])

            # projections: Q^T, K^T, V^T  [i, t]
            qt = big.tile([128, BT], f32r, tag="qt")
            kt = big.tile([128, BT], f32r, tag="kt")
            vt = big.tile([128, BT], f32, tag="vt")
            for w_sb, dst in ((wq_sb, qt), (wk_sb, kt), (wv_sb, vt)):
                for blk in range(BT // 512):
                    pp = ps_mm.tile([128, 512], f32, tag="mm")
                    nc.tensor.matmul(
                        pp[:],
                        w_sb[:],
                        xt[:, 512 * blk : 512 * (blk + 1)],
                        start=True,
                        stop=True,
                    )
                    nc.vector.tensor_copy(dst[:, 512 * blk : 512 * (blk + 1)], pp[:])

            # V back to [s, j] layout, chunk c at cols c*128
            v_sb = big.tile([128, BT], f32r, tag="v")
            for c in range(NT):
                pt = ps_mm.tile([128, 128], f32, tag="mm")
                nc.tensor.transpose(pt[:], vt[:, 128 * c : 128 * (c + 1)], ident[:])
                nc.vector.tensor_copy(v_sb[:, 128 * c : 128 * (c + 1)], pt[:])

            # attention
            y_sb = big.tile([128, BT], f32r, tag="y")
            for b in range(B):
                for tb in range(T // TB):
                    tcol = b * T + tb * TB
                    py = ps_y.tile([128, TB], f32, tag="y")
                    psum = ps_sum.tile([128, TB], f32, tag="sum")
                    for s in range(NS):
                        scol = b * T + s * 128
                        ps = ps_s.tile([128, TB], f32, tag="s")
                        nc.tensor.matmul(
                            ps[:],
                            kt[:, scol : scol + 128],
                            qt[:, tcol : tcol + TB],
                            start=True,
                            stop=True,
                        )
                        e_sb = work.tile([128, TB], f32r, tag="e")
                        nc.scalar.activation(e_sb[:], ps[:], Exp, scale=float(SCALE))
                        nc.tensor.matmul(
                            psum[:],
                            ones[:],
                            e_sb[:],
                            start=(s == 0),
                            stop=(s == NS - 1),
                            skip_group_check=True,
                        )
                        nc.tensor.matmul(
                            py[:],
                            v_sb[:, scol : scol + 128],
                            e_sb[:],
                            start=(s == 0),
                            stop=(s == NS - 1),
                            skip_group_check=True,
                        )
                    r_sb = work.tile([128, TB], f32, tag="r")
                    nc.vector.reciprocal(r_sb[:], psum[:])
                    nc.vector.tensor_mul(y_sb[:, tcol : tcol + TB], py[:], r_sb[:])

            # unify: out^T = Wu_h^T @ Y^T
            out_sb = big.tile([128, BT], f32, tag="out")
            for blk in range(BT // 512):
                po = ps_mm.tile([128, 512], f32, tag="mm")
                nc.tensor.matmul(
                    po[:],
                    wu_sb[:],
                    y_sb[:, 512 * blk : 512 * (blk + 1)],
                    start=True,
                    stop=True,
                )
                nc.vector.tensor_copy(out_sb[:, 512 * blk : 512 * (blk + 1)], po[:])
                nc.sync.dma_start(out_d[:, 512 * blk : 512 * (blk + 1)],
                                  out_sb[:, 512 * blk : 512 * (blk + 1)])

    nc.compile()
    return nc


def _get_nc():
    global _compiled
    if _compiled is None:
        _compiled = _build()
    return _compiled


def kernel(x, Wq, Wk, Wv, Wu, bu, **_run_kwargs):
    from concourse.bass_utils import run_bass_kernel_spmd

    nc = _get_nc()

    x = np.ascontiguousarray(np.asarray(x, dtype=np.float32).reshape(BT, K))
    Wq = np.asarray(Wq, dtype=np.float32)
    Wk = np.asarray(Wk, dtype=np.float32)
    Wv = np.asarray(Wv, dtype=np.float32)
    Wu = np.asarray(Wu, dtype=np.float32)
    bu = np.asarray(bu, dtype=np.float32)

    in_maps = []
    for c in range(NCORES):
        sl = slice(c * K, (c + 1) * K)
        in_maps.append(
            {
                "x": x,
                "wq": np.ascontiguousarray(Wq[:, sl]),
                "wk": np.ascontiguousarray(Wk[:, sl]),
                "wv": np.ascontiguousarray(Wv[:, sl]),
                "wu": np.ascontiguousarray(Wu[sl, :]),
            }
        )

    res = run_bass_kernel_spmd(nc, in_maps, list(range(NCORES)), **_run_kwargs)

    out = np.zeros((BT, K), dtype=np.float32)
    for c in range(NCORES):
        out += res.results[c]["out"].T
    out += bu[None, :]
    result = out.reshape(B, T, K)
    if _run_kwargs:
        return result, res
    return result
